# revision 5
# baseline (speedup 1.0000x reference)
"""AttentivePool Trainium2 kernel, v3: length-specialized programs.

The reference mask is a valid-length prefix mask (every sample has
len >= T/2 valid leading frames).  kernel() reads the actual lengths from
the mask at runtime and JIT-specializes one Bass program per core pair of
samples, with every O(T) loop sliced to [0, len).  This removes the mask
broadcast/multiply passes, the -30 softmax-bias preload matmuls, and cuts
all DVE/ACT/PE/DMA passes by ~25% on average.  Samples are paired
longest-with-shortest across cores so per-core work is balanced.

Per-sample flow (x [C=1536, T], l = valid length, chunks of 128 ch):
  ph1   per chunk: DMA x[:, :l];  mm1 partial (PE, 512-col banks);
        DVE tensor_scalar accum -> sum x;  ACT Square accum -> sum x^2
  ph2   mu = sx/l, sd = sqrt(sx2/l - mu^2); cb = w1b@mu + w1c@sd + b1 (PE)
  ph3   h = h_ps + cb -> bf16; PE-transpose 128-blocks; bn_stats LN;
        (h-mu)*rstd per block; relu; PE-transpose back; ACT Tanh from
        PSUM -> th bf16
  ph4   per chunk: mm2 a = w2@th (PE); ACT Exp halves (accum -> den);
        DVE e*x (accum -> sex); DVE ex*x (accum -> sx2)
  ph5   mu2 = sex/den, sd2 = sqrt(sx2/den - mu2^2) -> out

Non-prefix masks fall back to a general T-wide program (the previous
kernel, kept verbatim below).
"""

import numpy as np

B, C, T = 16, 1536, 2000
ATTN = 128
NCORES = 8
BPC = B // NCORES          # samples per core
NCH = C // 128             # 12 channel chunks
MASK_NEG = -30.0

_CACHE = {}


def _ceil128(n):
    return (n + 127) // 128 * 128


def _banks(l, width=512):
    out = []
    o = 0
    while o < l:
        out.append((o, min(width, l - o)))
        o += width
    return out


def _split_waits(nc, max_waits=1):
    """walrus in this toolchain rejects >1 sync-wait per instruction; hoist
    excess waits onto injected same-engine NOPs that run just before."""
    from concourse import mybir
    ctr = 0
    for fn in nc.m.functions:
        for blk in fn.blocks:
            out = []
            changed = False
            for ins in blk.instructions:
                si = ins.sync_info
                ow = list(si.on_wait) if si and si.on_wait else []
                if len(ow) > max_waits:
                    changed = True
                    excess = ow[:-max_waits]
                    for i in range(0, len(excess), max_waits):
                        ctr += 1
                        out.append(mybir.InstNoOp(
                            name=f"wsplit_{ctr}", engine=ins.engine,
                            ins=[], outs=[],
                            sync_info=mybir.SyncInfo(
                                on_wait=excess[i:i + max_waits],
                                on_update=[])))
                    si.on_wait = ow[-max_waits:]
                    ins.sync_info = si
                out.append(ins)
            if changed:
                blk.instructions = out


def _build_nc_lens(lens, k_sq_dve=(4, 2), pool_sumx=False, pool_ph4=(False, False),
                   hT_copy_dve=True, lead_chunks=4, half_products=True,
                   pool_e2=(False, False)):
    """Length-specialized per-core program. lens = (l0, l1).

    k_sq_dve[s]: chunks per sample whose sum(x^2) runs as DVE TT+TS instead
    of ACT Square (ACT/DVE balance).  pool_sumx: run the sum(x) accums on
    the (otherwise idle) GpSimd engine.  k_ph4_pool: chunks per sample whose
    phase-4 sex accum runs on GpSimd.
    """
    import contextlib

    import concourse.bass as bass
    import concourse.tile as tile
    from concourse import mybir

    fp32 = mybir.dt.float32
    bf16 = mybir.dt.bfloat16
    AF = mybir.ActivationFunctionType
    OP = mybir.AluOpType

    lmax = max(lens)

    nc = bass.Bass("TRN2", target_bir_lowering=False)

    x_d = nc.dram_tensor("x", [BPC, C, T], bf16, kind="ExternalInput")
    w1aT_d = nc.dram_tensor("w1aT", [128, NCH, 128], bf16, kind="ExternalInput")
    w1bT_d = nc.dram_tensor("w1bT", [128, NCH, 128], bf16, kind="ExternalInput")
    w1cT_d = nc.dram_tensor("w1cT", [128, NCH, 128], bf16, kind="ExternalInput")
    w2T_d = nc.dram_tensor("w2T", [128, NCH, 128], bf16, kind="ExternalInput")
    cst_d = nc.dram_tensor("cst", [128, 16], fp32, kind="ExternalInput")
    id_d = nc.dram_tensor("ident", [128, 128], bf16, kind="ExternalInput")
    out_d = nc.dram_tensor("out", [BPC, 2 * C], fp32, kind="ExternalOutput")

    with tile.TileContext(nc) as tc:
        with contextlib.ExitStack() as ctx:
            consts = ctx.enter_context(tc.tile_pool(name="consts", bufs=1))
            xpool = ctx.enter_context(tc.tile_pool(name="xres", bufs=2))
            scr = ctx.enter_context(tc.tile_pool(name="scr", bufs=10))
            hpool = ctx.enter_context(tc.tile_pool(name="hbuf", bufs=2))
            stpool = ctx.enter_context(tc.tile_pool(name="stats", bufs=2))
            psum = ctx.enter_context(tc.tile_pool(name="ps", bufs=2, space="PSUM"))
            psa = ctx.enter_context(tc.tile_pool(name="psa", bufs=2, space="PSUM"))

            # --- constants (loaded once) ---
            w1aT = consts.tile([128, NCH, 128], bf16)
            w1bT = consts.tile([128, NCH, 128], bf16)
            w1cT = consts.tile([128, NCH, 128], bf16)
            w2T = consts.tile([128, NCH, 128], bf16)
            cst = consts.tile([128, 16], fp32)
            ident = consts.tile([128, 128], bf16)
            b1c = cst[:, 0:1]
            eps_c = cst[:, 13:14]

            LS = [int(lens[s]) for s in range(BPC)]
            LPAD = [_ceil128(l) for l in LS]
            HALVES = []
            for l in LS:
                hv = [(0, min(1024, l))]
                if l > 1024:
                    hv.append((1024, l - 1024))
                HALVES.append(hv)

            sv_t, xts, hps_t, th_t = [], [], [], []

            def sq_on_dve(s, i):
                k = k_sq_dve[s]
                if k <= 0:
                    return False
                step = NCH // k
                return i % step == 0 and i // step < k

            # ---------- phase 1: DMA emission and stats emission ----------
            for s in range(BPC):
                sv = stpool.tile([128, 360], fp32, tag="sv")
                sv_t.append(sv)
                xts.append([])

            def emit_dma(s):
                l = LS[s]
                for i in range(NCH):
                    xb = xpool.tile([128, l], bf16, tag=f"x{i}")
                    xts[s].append(xb)
                    nc.sync.dma_start(out=xb,
                                      in_=x_d[s, i * 128:(i + 1) * 128, 0:l])
                    if s == 0 and i == 2:
                        # weights after the first three x chunks: stats can
                        # start immediately; mm1 needs w1aT only once chunk
                        # 0 has landed anyway
                        for t_, d_ in ((w1aT, w1aT_d), (w1bT, w1bT_d),
                                       (w1cT, w1cT_d), (w2T, w2T_d),
                                       (cst, cst_d), (ident, id_d)):
                            nc.sync.dma_start(out=t_, in_=d_[:])

            def emit_stats(s):
                l = LS[s]
                sv = sv_t[s]
                sumx = sv[:, 144:144 + NCH]
                sumq = sv[:, 156:156 + NCH]
                for i in range(NCH):
                    xb = xts[s][i]
                    sc1 = scr.tile([128, lmax], bf16, tag="big")
                    sumx_eng = nc.gpsimd if (pool_sumx and s == 1) else nc.vector
                    sumx_eng.tensor_scalar(out=sc1[:, 0:l], in0=xb,
                                           scalar1=1.0,
                                           scalar2=0.0, op0=OP.mult,
                                           op1=OP.add,
                                           accum_out=sumx[:, i:i + 1])
                    sc2 = scr.tile([128, lmax], bf16, tag="big")
                    if sq_on_dve(s, i):
                        nc.vector.tensor_mul(out=sc2[:, 0:l], in0=xb, in1=xb)
                        nc.vector.tensor_scalar(out=sc2[:, 0:l],
                                                in0=sc2[:, 0:l],
                                                scalar1=1.0, scalar2=0.0,
                                                op0=OP.mult, op1=OP.add,
                                                accum_out=sumq[:, i:i + 1])
                    else:
                        nc.scalar.activation(out=sc2[:, 0:l], in_=xb,
                                             func=AF.Square,
                                             accum_out=sumq[:, i:i + 1])

            # ---------- per sample: mm1, ph2, LN ----------
            def emit_mid(s):
                l, lpad = LS[s], LPAD[s]
                nblk = lpad // 128
                rl = 1.0 / float(l)
                sv = sv_t[s]
                var_a = sv[:, 6 * NCH:7 * NCH]
                t0_a = sv[:, 7 * NCH:8 * NCH]
                sumx = sv[:, 144:144 + NCH]
                sumq = sv[:, 156:156 + NCH]
                cb = sv[:, 192:193]
                mv = sv[:, 208:240].rearrange("p (j two) -> p j two", two=2)
                rstd = sv[:, 240:256]
                st6 = sv[:, 256:256 + 48].rearrange("p (j k) -> p j k", k=6)
                st6b = sv[:, 144:192].rearrange("p (j k) -> p j k", k=6)

                # mm1 in 1024-col halves so each PSUM accumulator is 2 banks
                h_ps = []
                hps_t.append(h_ps)
                for (ho, hn) in HALVES[s]:
                    hp = psum.tile([128, 1024], fp32, tag="mm")
                    h_ps.append((ho, hn, hp))
                    for i in range(NCH):
                        for (o, n) in _banks(hn):
                            nc.tensor.matmul(
                                hp[:, o:o + n],
                                lhsT=w1aT[:, i, :],
                                rhs=xts[s][i][:, ho + o:ho + o + n],
                                start=(i == 0), stop=(i == NCH - 1))

                # ---------- phase 2: mu/sd (bf16, short chain), cb ----------
                mu_b = stpool.tile([128, 2 * NCH], bf16, tag="mub")
                nc.vector.tensor_scalar(out=mu_b[:, 0:NCH], in0=sumx,
                                        scalar1=rl, scalar2=None, op0=OP.mult)
                nc.vector.tensor_mul(out=t0_a, in0=mu_b[:, 0:NCH],
                                     in1=mu_b[:, 0:NCH])
                nc.vector.scalar_tensor_tensor(out=var_a, in0=sumq, scalar=rl,
                                               in1=t0_a, op0=OP.mult,
                                               op1=OP.subtract)
                nc.vector.tensor_scalar(out=var_a, in0=var_a, scalar1=1e-9,
                                        scalar2=None, op0=OP.max)
                nc.scalar.activation(out=mu_b[:, NCH:2 * NCH], in_=var_a,
                                     func=AF.Sqrt)
                cb_ps = psa.tile([128, 2048], bf16, tag="a")
                cb_ps32 = cb_ps.bitcast(fp32)
                for i in range(NCH):
                    nc.tensor.matmul(cb_ps32[:, 0:1], lhsT=w1bT[:, i, :],
                                     rhs=mu_b[:, i:i + 1],
                                     start=(i == 0), stop=False)
                for i in range(NCH):
                    nc.tensor.matmul(cb_ps32[:, 0:1], lhsT=w1cT[:, i, :],
                                     rhs=mu_b[:, NCH + i:NCH + i + 1],
                                     start=False, stop=(i == NCH - 1))
                nc.vector.tensor_tensor(out=cb, in0=cb_ps32[:, 0:1],
                                         in1=b1c, op=OP.add)

                # ---------- phase 3: h -> LN -> relu -> (T) tanh -> th ------
                h_sb = hpool.tile([128, lpad], bf16, tag="hsb")
                for (ho, hn, hp) in h_ps:
                    nc.scalar.activation(out=h_sb[:, ho:ho + hn],
                                         in_=hp[:, 0:hn],
                                         func=AF.Identity, bias=cb)
                if lpad > l:
                    nc.vector.memset(h_sb[:, l:lpad], 0.0)

                tp_ps = psum.tile([128, 2048], bf16, tag="mm")
                for j in range(nblk):
                    nc.tensor.transpose(
                        tp_ps[:, j * 128:(j + 1) * 128],
                        in_=h_sb[:, j * 128:(j + 1) * 128],
                        identity=ident)

                for j in range(nblk):
                    stt = st6 if j < 8 else st6b
                    nc.vector.bn_stats(out=stt[:, j % 8, :],
                                       in_=tp_ps[:, j * 128:(j + 1) * 128])
                    nc.vector.bn_aggr(out=mv[:, j, :], in_=stt[:, j % 8, :])
                for (g0, g1) in ((0, min(8, nblk)), (8, nblk)):
                    if g0 >= g1:
                        continue
                    nc.scalar.activation(out=rstd[:, g0:g1],
                                         in_=mv[:, g0:g1, 1],
                                         func=AF.Sqrt, bias=eps_c)
                    nc.vector.reciprocal(out=rstd[:, g0:g1],
                                         in_=rstd[:, g0:g1])

                thT = hpool.tile([128, lpad], bf16, tag="thT")
                for j in range(nblk):
                    blk = slice(j * 128, (j + 1) * 128)
                    nc.vector.tensor_scalar(
                        out=thT[:, blk], in0=tp_ps[:, blk],
                        scalar1=mv[:, j, 0:1], scalar2=rstd[:, j:j + 1],
                        op0=OP.subtract, op1=OP.mult)
                nc.vector.tensor_scalar(out=thT, in0=thT, scalar1=0.0,
                                        scalar2=None, op0=OP.max)

                th = hpool.tile([128, lpad], bf16, tag="th")
                th_t.append(th)
                tb_ps = psum.tile([128, 2048], bf16, tag="mm")
                for j in range(nblk):
                    nc.tensor.transpose(
                        tb_ps[:, j * 128:(j + 1) * 128],
                        in_=thT[:, j * 128:(j + 1) * 128],
                        identity=ident)
                for (co, cn) in _banks(lpad, 512):
                    nc.scalar.activation(out=th[:, co:co + cn],
                                         in_=tb_ps[:, co:co + cn],
                                         func=AF.Tanh)

            def emit_ph4_chunk(s, i, use_mm=False):
                l = LS[s]
                sv = sv_t[s]
                denh = sv[:, 168:168 + 2 * NCH]
                sexh = sv[:, 304:304 + 2 * NCH]
                sx2h = sv[:, 328:328 + 2 * NCH]
                if s == 0:
                    acc1 = nc.gpsimd if (pool_ph4[0] and i % 2 == 0) else nc.vector
                    acc2 = nc.gpsimd if pool_ph4[0] else nc.vector
                else:
                    acc1 = nc.gpsimd if (pool_ph4[1] and i >= 6) else nc.vector
                    acc2 = nc.vector
                e_i = scr.tile([128, lmax], bf16, tag="big")
                ex_i = scr.tile([128, lmax], bf16, tag="big")
                e2_i = scr.tile([128, lmax], bf16, tag="big")
                for hf, (ho, hn) in enumerate(HALVES[s]):
                    if use_mm and hf % 2 == 1:
                        a_ps = psum.tile([128, 2048], bf16, tag="mm")
                    else:
                        a_ps = psa.tile([128, 2048], bf16, tag="a")
                    a_ps32 = a_ps.bitcast(fp32)
                    for (o, n) in _banks(hn):
                        nc.tensor.matmul(
                            a_ps32[:, o:o + n],
                            lhsT=w2T[:, i, :],
                            rhs=th_t[s][:, ho + o:ho + o + n],
                            start=True, stop=True)
                    nc.scalar.activation(
                        out=e_i[:, ho:ho + hn], in_=a_ps32[:, 0:hn],
                        func=AF.Exp,
                        accum_out=denh[:, hf * NCH + i:hf * NCH + i + 1])
                    if not half_products:
                        continue
                    hsl = slice(ho, ho + hn)
                    nc.vector.tensor_mul(out=ex_i[:, hsl], in0=e_i[:, hsl],
                                         in1=xts[s][i][:, hsl])
                    mul2 = nc.gpsimd if pool_e2[s] else nc.vector
                    mul2.tensor_mul(out=e2_i[:, hsl], in0=ex_i[:, hsl],
                                    in1=xts[s][i][:, hsl])
                    # e_i half is dead after the first product; reuse it as
                    # the accum-pass output so nothing serializes on ex_i
                    acc1.tensor_scalar(out=ex_i[:, hsl], in0=ex_i[:, hsl],
                                       scalar1=1.0, scalar2=0.0,
                                       op0=OP.mult, op1=OP.add,
                                       accum_out=sexh[:, hf * NCH + i:
                                                      hf * NCH + i + 1])
                    acc2.tensor_scalar(out=e2_i[:, hsl], in0=e2_i[:, hsl],
                                       scalar1=1.0, scalar2=0.0,
                                       op0=OP.mult, op1=OP.add,
                                       accum_out=sx2h[:, hf * NCH + i:
                                                      hf * NCH + i + 1])
                if not half_products:
                    sl = slice(0, l)
                    nc.vector.tensor_mul(out=ex_i[:, sl], in0=e_i[:, sl],
                                         in1=xts[s][i])
                    mul2 = nc.gpsimd if pool_e2[s] else nc.vector
                    mul2.tensor_mul(out=e2_i[:, sl], in0=ex_i[:, sl],
                                    in1=xts[s][i])
                    acc1.tensor_scalar(out=ex_i[:, sl], in0=ex_i[:, sl],
                                       scalar1=1.0, scalar2=0.0,
                                       op0=OP.mult, op1=OP.add,
                                       accum_out=sexh[:, i:i + 1])
                    acc2.tensor_scalar(out=e2_i[:, sl], in0=e2_i[:, sl],
                                       scalar1=1.0, scalar2=0.0,
                                       op0=OP.mult, op1=OP.add,
                                       accum_out=sx2h[:, i:i + 1])

            # emission order: s0 dma+stats, s1 dma, s0 mid-pipe, s1 stats,
            # lead chunks of ph4(s0), s1 mid-pipe, interleaved remainder
            emit_dma(0)
            emit_stats(0)
            emit_dma(1)
            with tc.high_priority(offset=100000):
                emit_mid(0)
            emit_stats(1)
            with tc.high_priority(offset=70000):
                for i in range(min(lead_chunks, NCH)):
                    emit_ph4_chunk(0, i)
            emit_mid(1)
            # s0-solo prefix, then 1:1 alternate, then s1-only tail: s1 work
            # enters the in-order engine streams only once LN(s1) output
    # exists, avoiding head-of-line stalls
            solo = 8
            seq = [(0, i) for i in range(lead_chunks, solo)]
            j1 = 0
            for i in range(solo, NCH):
                seq.append((0, i))
                seq.append((1, j1))
                j1 += 1
            while j1 < NCH:
                seq.append((1, j1))
                j1 += 1
            for k, (s, i) in enumerate(seq):
                emit_ph4_chunk(s, i, use_mm=k % 2 == 1)

            # ---------- phase 5: outputs ----------
            for s in range(BPC):
                sv = sv_t[s]
                den_a = sv[:, 3 * NCH:4 * NCH]
                sex_a = sv[:, 4 * NCH:5 * NCH]
                sx2_a = sv[:, 5 * NCH:6 * NCH]
                t0_a = sv[:, 7 * NCH:8 * NCH]
                mu2_a = sv[:, 8 * NCH:9 * NCH]
                ms2_a = sv[:, 9 * NCH:10 * NCH]
                sd2_a = sv[:, 10 * NCH:11 * NCH]
                rden_a = sv[:, 11 * NCH:12 * NCH]
                denh = sv[:, 168:168 + 2 * NCH]
                sexh = sv[:, 304:304 + 2 * NCH]
                sx2h = sv[:, 328:328 + 2 * NCH]
                if len(HALVES[s]) > 1:
                    nc.vector.tensor_add(out=den_a, in0=denh[:, 0:NCH],
                                         in1=denh[:, NCH:2 * NCH])
                else:
                    nc.vector.tensor_scalar(out=den_a, in0=denh[:, 0:NCH],
                                            scalar1=1.0, scalar2=None,
                                            op0=OP.mult)
                if len(HALVES[s]) > 1 and half_products:
                    nc.vector.tensor_add(out=sex_a, in0=sexh[:, 0:NCH],
                                         in1=sexh[:, NCH:2 * NCH])
                    nc.vector.tensor_add(out=sx2_a, in0=sx2h[:, 0:NCH],
                                         in1=sx2h[:, NCH:2 * NCH])
                else:
                    nc.vector.tensor_scalar(out=sex_a, in0=sexh[:, 0:NCH],
                                            scalar1=1.0, scalar2=None,
                                            op0=OP.mult)
                    nc.vector.tensor_scalar(out=sx2_a, in0=sx2h[:, 0:NCH],
                                            scalar1=1.0, scalar2=None,
                                            op0=OP.mult)
                nc.vector.reciprocal(out=rden_a, in_=den_a)
                nc.vector.tensor_mul(out=mu2_a, in0=sex_a, in1=rden_a)
                nc.vector.tensor_mul(out=ms2_a, in0=sx2_a, in1=rden_a)
                nc.vector.tensor_mul(out=t0_a, in0=mu2_a, in1=mu2_a)
                nc.vector.tensor_tensor(out=ms2_a, in0=ms2_a, in1=t0_a,
                                        op=OP.subtract)
                nc.vector.tensor_scalar(out=ms2_a, in0=ms2_a, scalar1=1e-9,
                                        scalar2=None, op0=OP.max)
                nc.scalar.activation(out=sd2_a, in_=ms2_a, func=AF.Sqrt)

                nc.sync.dma_start(
                    out=out_d[s, 0:C].rearrange("(i p) -> p i", p=128),
                    in_=mu2_a)
                nc.sync.dma_start(
                    out=out_d[s, C:2 * C].rearrange("(i p) -> p i", p=128),
                    in_=sd2_a)

    _split_waits(nc)
    return nc


def _prep_weights_v3(w1, b1, w2):
    import ml_dtypes
    f = np.float32
    bf = ml_dtypes.bfloat16
    w1T = np.ascontiguousarray(np.asarray(w1, f).T)      # [3C, 128]
    w1aT = np.ascontiguousarray(
        w1T[0:C].reshape(NCH, 128, 128).transpose(1, 0, 2)).astype(bf)
    w1bT = np.ascontiguousarray(
        w1T[C:2 * C].reshape(NCH, 128, 128).transpose(1, 0, 2)).astype(bf)
    w1cT = np.ascontiguousarray(
        w1T[2 * C:3 * C].reshape(NCH, 128, 128).transpose(1, 0, 2)).astype(bf)
    w2T = np.ascontiguousarray(
        np.asarray(w2, f).reshape(NCH, 128, 128).transpose(2, 0, 1)).astype(bf)
    cst = np.zeros((128, 16), f)
    cst[:, 0] = np.asarray(b1, f)
    cst[:, 13] = 1e-5
    ident = np.eye(128, dtype=bf)
    return dict(w1aT=w1aT, w1bT=w1bT, w1cT=w1cT, w2T=w2T, cst=cst,
                ident=ident)


def _run_programs_on_devices(progs, in_maps):
    """Run per-core Bass programs, program i pinned to jax device i.

    Mirrors bass2jax.run_bass_via_pjrt's single-core path, with explicit
    device placement so the 8 distinct programs land on 8 distinct cores.
    """
    import jax
    from concourse import bass2jax, mybir

    bass2jax.install_neuronx_cc_hook()
    devices = jax.devices()
    futs = []
    metas = []
    for core, (nc, in_map) in enumerate(zip(progs, in_maps)):
        part_name = (nc.partition_id_tensor.name
                     if nc.partition_id_tensor else None)
        in_names, out_names, out_avals, zero_outs = [], [], [], []
        for alloc in nc.m.functions[0].allocations:
            if not isinstance(alloc, mybir.MemoryLocationSet):
                continue
            name = alloc.memorylocations[0].name
            if alloc.kind == "ExternalInput":
                if name != part_name:
                    in_names.append(name)
            elif alloc.kind == "ExternalOutput":
                out_names.append(name)
                shape = tuple(alloc.tensor_shape)
                dtype = mybir.dt.np(alloc.dtype)
                out_avals.append(jax.core.ShapedArray(shape, dtype))
                zero_outs.append(np.zeros(shape, dtype))
        n_params = len(in_names)
        all_names = list(in_names) + list(out_names)
        if part_name is not None:
            all_names.append(part_name)
        all_names = tuple(all_names)
        donate = tuple(range(n_params, n_params + len(out_names)))

        def _body(*args, _nc=nc, _avals=tuple(out_avals), _names=all_names,
                  _onames=tuple(out_names), _part=part_name):
            operands = list(args)
            if _part is not None:
                operands.append(bass2jax.partition_id_tensor())
            outs = bass2jax._bass_exec_p.bind(
                *operands,
                out_avals=_avals,
                in_names=_names,
                out_names=_onames,
                lowering_input_output_aliases=(),
                sim_require_finite=True,
                sim_require_nnan=True,
                nc=_nc,
            )
            return tuple(outs)

        args = [np.asarray(in_map[nm]) for nm in in_names] + zero_outs
        dev = devices[core % len(devices)]
        with jax.default_device(dev):
            out_arrs = jax.jit(_body, donate_argnums=donate,
                               keep_unused=True)(*args)
        futs.append(out_arrs)
        metas.append(out_names)
    results = []
    for out_arrs, out_names in zip(futs, metas):
        results.append({nm: np.asarray(a) for nm, a in zip(out_names, out_arrs)})
    return results


def _kernel_prefix(x, mask, lens, w1, b1, w2, b2):
    import ml_dtypes

    order = np.argsort(-lens, kind="stable")          # longest first
    pairs = [(int(order[i]), int(order[B - 1 - i])) for i in range(NCORES)]

    wts = _prep_weights_v3(w1, b1, w2)
    xf = np.asarray(x, np.float32).astype(ml_dtypes.bfloat16)

    progs, in_maps = [], []
    for (sa, sb) in pairs:
        key = ("lens", int(lens[sa]), int(lens[sb]))
        if key not in _CACHE:
            _CACHE[key] = _build_nc_lens((int(lens[sa]), int(lens[sb])))
        progs.append(_CACHE[key])
        m = {"x": np.ascontiguousarray(xf[[sa, sb]])}
        m.update(wts)
        in_maps.append(m)

    res = _run_programs_on_devices(progs, in_maps)
    out = np.zeros((B, 2 * C), np.float32)
    for ci, (sa, sb) in enumerate(pairs):
        o = res[ci]["out"].reshape(BPC, 2 * C)
        out[sa] = o[0]
        out[sb] = o[1]
    _CACHE["last_progs"] = progs
    return out


# ---------------------------------------------------------------------------
# General fallback (arbitrary 0/1 masks): previous full-T kernel, verbatim.
# ---------------------------------------------------------------------------

TPAD = 2048
NTT = TPAD // 128
BANKS = [(0, 512), (512, 512), (1024, 512), (1536, 464)]


def _build_nc_general(trivial_ln=True, trivial_b2=True):
    import concourse.bass as bass
    import concourse.tile as tile
    from concourse import mybir

    fp32 = mybir.dt.float32
    AF = mybir.ActivationFunctionType
    OP = mybir.AluOpType

    nc = bass.Bass("TRN2", target_bir_lowering=False)

    bf16 = mybir.dt.bfloat16
    x_d = nc.dram_tensor("x", [BPC, C, T], bf16, kind="ExternalInput")
    mk_d = nc.dram_tensor("maskb", [BPC, T], bf16, kind="ExternalInput")
    mbias_d = nc.dram_tensor("mbias", [BPC, T], bf16, kind="ExternalInput")
    w1aT_d = nc.dram_tensor("w1aT", [128, NCH, 128], bf16, kind="ExternalInput")
    w1bT_d = nc.dram_tensor("w1bT", [128, NCH, 128], fp32, kind="ExternalInput")
    w1cT_d = nc.dram_tensor("w1cT", [128, NCH, 128], fp32, kind="ExternalInput")
    w2T_d = nc.dram_tensor("w2T", [128, NCH, 128], bf16, kind="ExternalInput")
    gb_d = nc.dram_tensor("gb", [128, 128], fp32, kind="ExternalInput")
    bb_d = nc.dram_tensor("bb", [128, 128], fp32, kind="ExternalInput")
    cst_d = nc.dram_tensor("cst", [128, 16], fp32, kind="ExternalInput")
    ones_d = nc.dram_tensor("ones_row", [1, 128], bf16, kind="ExternalInput")
    id_d = nc.dram_tensor("ident", [128, 128], fp32, kind="ExternalInput")
    out_d = nc.dram_tensor("out", [BPC, 2 * C], fp32, kind="ExternalOutput")

    with tile.TileContext(nc) as tc:
        import contextlib
        with contextlib.ExitStack() as ctx:
            consts = ctx.enter_context(tc.tile_pool(name="consts", bufs=1))
            xpool = ctx.enter_context(tc.tile_pool(name="xres", bufs=1))
            nmpool = ctx.enter_context(tc.tile_pool(name="nm", bufs=2))
            scr = ctx.enter_context(tc.tile_pool(name="scr", bufs=4))
            hpool = ctx.enter_context(tc.tile_pool(name="hbuf", bufs=2))
            stpool = ctx.enter_context(tc.tile_pool(name="stats", bufs=2))
            psum = ctx.enter_context(tc.tile_pool(name="ps", bufs=1, space="PSUM"))
            psa = ctx.enter_context(tc.tile_pool(name="psa", bufs=2, space="PSUM"))

            w1aT = consts.tile([128, NCH, 128], bf16)
            w1bT = consts.tile([128, NCH, 128], fp32)
            w1cT = consts.tile([128, NCH, 128], fp32)
            w2T = consts.tile([128, NCH, 128], bf16)
            gb = consts.tile([128, 128], fp32)
            bb = consts.tile([128, 128], fp32)
            cst = consts.tile([128, 16], fp32)
            ones_row = consts.tile([1, 128], bf16)
            ident = consts.tile([128, 128], fp32)
            for t_, d_ in ((w1aT, w1aT_d), (w1bT, w1bT_d), (w1cT, w1cT_d),
                           (w2T, w2T_d), (gb, gb_d), (bb, bb_d), (cst, cst_d),
                           (ones_row, ones_d), (ident, id_d)):
                nc.sync.dma_start(out=t_, in_=d_[:])
            b1c = cst[:, 0:1]
            b2c = cst[:, 1:1 + NCH]
            eps_c = cst[:, 13:14]

            for s in range(BPC):
                mb_row = stpool.tile([1, T], bf16, tag="mbrow")
                nc.sync.dma_start(out=mb_row, in_=mbias_d[s, :][None, :])

                sv = stpool.tile([128, 360], fp32, tag="sv")
                mu_a = sv[:, 0 * NCH:1 * NCH]
                q_a = sv[:, 1 * NCH:2 * NCH]
                sd_a = sv[:, 2 * NCH:3 * NCH]
                den_a = sv[:, 3 * NCH:4 * NCH]
                sex_a = sv[:, 4 * NCH:5 * NCH]
                sx2_a = sv[:, 5 * NCH:6 * NCH]
                var_a = sv[:, 6 * NCH:7 * NCH]
                t0_a = sv[:, 7 * NCH:8 * NCH]
                mu2_a = sv[:, 8 * NCH:9 * NCH]
                ms2_a = sv[:, 9 * NCH:10 * NCH]
                sd2_a = sv[:, 10 * NCH:11 * NCH]
                rden_a = sv[:, 11 * NCH:12 * NCH]
                msum = sv[0:1, 144:145]
                L_b = sv[:, 145:146]
                rcpL_b = sv[:, 146:147]
                cb = sv[:, 147:148]
                mv = sv[:, 160:192].rearrange("p (j two) -> p j two", two=2)
                rstd = sv[:, 192:208]
                st6 = sv[:, 208:304].rearrange("p (j k) -> p j k", k=6)

                nm_b = nmpool.tile([128, T], bf16, tag="nmb")
                nc.sync.dma_start(
                    out=nm_b, in_=mk_d[s, :][None, :].to_broadcast((128, T)))
                nc.scalar.activation(out=nm_b, in_=nm_b, func=AF.Copy,
                                     accum_out=L_b)
                nc.vector.reciprocal(out=rcpL_b, in_=L_b)
                nc.vector.tensor_scalar(out=nm_b, in0=nm_b, scalar1=rcpL_b,
                                        scalar2=None, op0=OP.mult)

                h_ps = psum.tile([128, TPAD], fp32, tag="mm")
                xt = []
                for i in range(NCH):
                    xb = xpool.tile([128, T], bf16, tag=f"x{i}")
                    xt.append(xb)
                    nc.sync.dma_start(out=xb, in_=x_d[s, i * 128:(i + 1) * 128, :])
                    for (o, n) in BANKS:
                        nc.tensor.matmul(
                            h_ps[:, o:o + n],
                            lhsT=w1aT[:, i, :],
                            rhs=xb[:, o:o + n],
                            start=(i == 0), stop=(i == NCH - 1))
                    xnm = scr.tile([128, T], bf16, tag="big")
                    nc.vector.tensor_mul(out=xnm, in0=xb, in1=nm_b)
                    nc.vector.tensor_scalar(out=xnm, in0=xnm, scalar1=1.0,
                                            scalar2=0.0, op0=OP.mult,
                                            op1=OP.add,
                                            accum_out=mu_a[:, i:i + 1])
                    nc.scalar.activation(out=xnm, in_=xnm, func=AF.Square,
                                         accum_out=q_a[:, i:i + 1])

                nc.vector.tensor_scalar(out=q_a, in0=q_a, scalar1=L_b,
                                        scalar2=None, op0=OP.mult)
                nc.vector.tensor_mul(out=t0_a, in0=mu_a, in1=mu_a)
                nc.vector.tensor_tensor(out=var_a, in0=q_a, in1=t0_a,
                                        op=OP.subtract)
                nc.vector.tensor_scalar(out=var_a, in0=var_a, scalar1=1e-9,
                                        scalar2=None, op0=OP.max)
                nc.scalar.activation(out=sd_a, in_=var_a, func=AF.Sqrt)

                cb_ps = psa.tile([128, 1], fp32, tag="a")
                for i in range(NCH):
                    nc.tensor.matmul(cb_ps, lhsT=w1bT[:, i, :],
                                     rhs=mu_a[:, i:i + 1],
                                     start=(i == 0), stop=False)
                for i in range(NCH):
                    nc.tensor.matmul(cb_ps, lhsT=w1cT[:, i, :],
                                     rhs=sd_a[:, i:i + 1],
                                     start=False, stop=(i == NCH - 1))
                nc.scalar.activation(out=cb, in_=cb_ps, func=AF.Identity,
                                     bias=b1c)

                h_sb = hpool.tile([128, TPAD], fp32, tag="hbuf")
                nc.scalar.activation(out=h_sb[:, 0:T], in_=h_ps[:, 0:T],
                                     func=AF.Identity, bias=cb)
                nc.vector.memset(h_sb[:, T:TPAD], 0.0)

                hT = hpool.tile([128, TPAD], fp32, tag="hbuf")
                for g in range(2):
                    tp_ps = psa.tile([128, 1024], fp32, tag="a")
                    for j in range(8):
                        jj = g * 8 + j
                        nc.tensor.transpose(tp_ps[:, j * 128:(j + 1) * 128],
                                            in_=h_sb[:, jj * 128:(jj + 1) * 128],
                                            identity=ident)
                    nc.scalar.activation(out=hT[:, g * 1024:(g + 1) * 1024],
                                         in_=tp_ps, func=AF.Copy)

                for j in range(NTT):
                    nc.vector.bn_stats(out=st6[:, j, :],
                                       in_=hT[:, j * 128:(j + 1) * 128])
                    nc.vector.bn_aggr(out=mv[:, j, :], in_=st6[:, j, :])
                nc.scalar.activation(out=rstd, in_=mv[:, :, 1], func=AF.Sqrt,
                                     bias=eps_c)
                nc.vector.reciprocal(out=rstd, in_=rstd)

                thT = hpool.tile([128, TPAD], fp32, tag="hbuf")
                for j in range(NTT):
                    blk = slice(j * 128, (j + 1) * 128)
                    nc.vector.tensor_scalar(
                        out=thT[:, blk], in0=hT[:, blk],
                        scalar1=mv[:, j, 0:1], scalar2=rstd[:, j:j + 1],
                        op0=OP.subtract, op1=OP.mult)
                    if not trivial_ln:
                        nc.vector.tensor_mul(out=thT[:, blk],
                                             in0=thT[:, blk], in1=gb)
                        nc.vector.tensor_add(out=thT[:, blk],
                                             in0=thT[:, blk], in1=bb)
                nc.vector.tensor_scalar(out=thT, in0=thT, scalar1=0.0,
                                        scalar2=None, op0=OP.max)
                nc.scalar.activation(out=thT, in_=thT, func=AF.Tanh)

                th = hpool.tile([128, TPAD], bf16, tag="thbuf")
                for g in range(2):
                    tb_ps = psa.tile([128, 1024], fp32, tag="a")
                    for j in range(8):
                        jj = g * 8 + j
                        nc.tensor.transpose(tb_ps[:, j * 128:(j + 1) * 128],
                                            in_=thT[:, jj * 128:(jj + 1) * 128],
                                            identity=ident)
                    nc.scalar.activation(out=th[:, g * 1024:(g + 1) * 1024],
                                         in_=tb_ps, func=AF.Copy)

                denh = stpool.tile([128, 2 * NCH], fp32, tag="denh")
                for i in range(NCH):
                    e_i = scr.tile([128, T], bf16, tag="big")
                    for hf, (ho, hws) in enumerate(
                            ((0, ((0, 512), (512, 512))),
                             (1024, ((0, 512), (512, 464))))):
                        a_ps = psa.tile([128, 1024], fp32, tag="a")
                        for (o, n) in hws:
                            nc.tensor.matmul(
                                a_ps[:, o:o + n],
                                lhsT=ones_row,
                                rhs=mb_row[:, ho + o:ho + o + n],
                                start=True, stop=False)
                            nc.tensor.matmul(
                                a_ps[:, o:o + n],
                                lhsT=w2T[:, i, :],
                                rhs=th[:, ho + o:ho + o + n],
                                start=False, stop=True)
                        hn = 1024 if hf == 0 else T - 1024
                        kw = {} if trivial_b2 else {"bias": b2c[:, i:i + 1]}
                        nc.scalar.activation(
                            out=e_i[:, ho:ho + hn], in_=a_ps[:, 0:hn],
                            func=AF.Exp,
                            accum_out=denh[:, hf * NCH + i:hf * NCH + i + 1],
                            **kw)
                    ex_i = scr.tile([128, T], bf16, tag="big")
                    nc.vector.tensor_mul(out=ex_i, in0=e_i, in1=xt[i])
                    nc.vector.tensor_scalar(out=ex_i, in0=ex_i, scalar1=1.0,
                                            scalar2=0.0, op0=OP.mult,
                                            op1=OP.add,
                                            accum_out=sex_a[:, i:i + 1])
                    nc.vector.tensor_mul(out=e_i, in0=ex_i, in1=xt[i])
                    nc.vector.tensor_scalar(out=e_i, in0=e_i, scalar1=1.0,
                                            scalar2=0.0, op0=OP.mult,
                                            op1=OP.add,
                                            accum_out=sx2_a[:, i:i + 1])

                nc.vector.tensor_add(out=den_a, in0=denh[:, 0:NCH],
                                     in1=denh[:, NCH:2 * NCH])
                nc.vector.reciprocal(out=rden_a, in_=den_a)
                nc.vector.tensor_mul(out=mu2_a, in0=sex_a, in1=rden_a)
                nc.vector.tensor_mul(out=ms2_a, in0=sx2_a, in1=rden_a)
                nc.vector.tensor_mul(out=t0_a, in0=mu2_a, in1=mu2_a)
                nc.vector.tensor_tensor(out=ms2_a, in0=ms2_a, in1=t0_a,
                                        op=OP.subtract)
                nc.vector.tensor_scalar(out=ms2_a, in0=ms2_a, scalar1=1e-9,
                                        scalar2=None, op0=OP.max)
                nc.scalar.activation(out=sd2_a, in_=ms2_a, func=AF.Sqrt)

                nc.sync.dma_start(
                    out=out_d[s, 0:C].rearrange("(i p) -> p i", p=128),
                    in_=mu2_a)
                nc.sync.dma_start(
                    out=out_d[s, C:2 * C].rearrange("(i p) -> p i", p=128),
                    in_=sd2_a)

    _split_waits(nc)
    return nc


def _prep_weights_general(w1, b1, ln_g, ln_b, w2, b2):
    f = np.float32
    import ml_dtypes
    bf = ml_dtypes.bfloat16
    w1T = np.ascontiguousarray(w1.T, dtype=f)
    w1aT = np.ascontiguousarray(
        w1T[0:C].reshape(NCH, 128, 128).transpose(1, 0, 2)).astype(bf)
    w1bT = np.ascontiguousarray(
        w1T[C:2 * C].reshape(NCH, 128, 128).transpose(1, 0, 2))
    w1cT = np.ascontiguousarray(
        w1T[2 * C:3 * C].reshape(NCH, 128, 128).transpose(1, 0, 2))
    w2T = np.ascontiguousarray(
        np.asarray(w2, f).reshape(NCH, 128, 128).transpose(2, 0, 1)).astype(bf)
    gb = np.ascontiguousarray(np.tile(np.asarray(ln_g, f)[None, :], (128, 1)))
    bb = np.ascontiguousarray(np.tile(np.asarray(ln_b, f)[None, :], (128, 1)))
    cst = np.zeros((128, 16), f)
    cst[:, 0] = np.asarray(b1, f)
    cst[:, 1:1 + NCH] = np.asarray(b2, f).reshape(NCH, 128).T
    cst[:, 13] = 1e-5
    ones_row = np.ones((1, 128), bf)
    ident = np.eye(128, dtype=f)
    return dict(w1aT=w1aT, w1bT=w1bT, w1cT=w1cT, w2T=w2T, gb=gb, bb=bb,
                cst=cst, ones_row=ones_row, ident=ident)


def _kernel_general(x, mask, w1, b1, ln_g, ln_b, w2, b2):
    from concourse.bass_utils import run_bass_kernel_spmd

    trivial_ln = bool(np.all(np.asarray(ln_g) == 1.0)
                      and np.all(np.asarray(ln_b) == 0.0))
    trivial_b2 = bool(np.all(np.asarray(b2) == 0.0))
    key = ("gen", trivial_ln, trivial_b2)
    if key not in _CACHE:
        _CACHE[key] = _build_nc_general(trivial_ln, trivial_b2)
    nc = _CACHE[key]

    wts = _prep_weights_general(w1, b1, ln_g, ln_b, w2, b2)
    import ml_dtypes
    xf = np.ascontiguousarray(
        np.asarray(x, np.float32).astype(ml_dtypes.bfloat16))
    mf = np.ascontiguousarray(np.asarray(mask, np.float32).reshape(B, T))
    maskb = np.ascontiguousarray(mf.astype(ml_dtypes.bfloat16))
    mbias = np.ascontiguousarray(
        ((mf - 1.0) * -MASK_NEG).astype(ml_dtypes.bfloat16))

    in_maps = []
    for c in range(NCORES):
        m = {"x": xf[c * BPC:(c + 1) * BPC],
             "maskb": maskb[c * BPC:(c + 1) * BPC],
             "mbias": mbias[c * BPC:(c + 1) * BPC]}
        m.update(wts)
        in_maps.append(m)

    res = run_bass_kernel_spmd(nc, in_maps, list(range(NCORES)))
    out = np.concatenate([res.results[c]["out"] for c in range(NCORES)], axis=0)
    _CACHE["last_progs"] = [nc]
    return out.reshape(B, 2 * C)


def kernel(x, mask, w1, b1, ln_g, ln_b, w2, b2):
    mask2 = np.asarray(mask).reshape(B, T)
    lens = mask2.sum(axis=-1).astype(np.int64)
    is_prefix = bool(
        np.all((np.arange(T)[None, :] < lens[:, None]) == (mask2 != 0)))
    trivial_ln = bool(np.all(np.asarray(ln_g) == 1.0)
                      and np.all(np.asarray(ln_b) == 0.0))
    trivial_b2 = bool(np.all(np.asarray(b2) == 0.0))
    if is_prefix and trivial_ln and trivial_b2 and np.all(lens >= 1):
        return _kernel_prefix(x, mask2, lens, w1, b1, w2, b2)
    return _kernel_general(x, mask, w1, b1, ln_g, ln_b, w2, b2)


# revision 6
# speedup vs baseline: 1.0029x; 1.0029x over previous
"""AttentivePool Trainium2 kernel, v3: length-specialized programs.

The reference mask is a valid-length prefix mask (every sample has
len >= T/2 valid leading frames).  kernel() reads the actual lengths from
the mask at runtime and JIT-specializes one Bass program per core pair of
samples, with every O(T) loop sliced to [0, len).  This removes the mask
broadcast/multiply passes, the -30 softmax-bias preload matmuls, and cuts
all DVE/ACT/PE/DMA passes by ~25% on average.  Samples are paired
longest-with-shortest across cores so per-core work is balanced.

Per-sample flow (x [C=1536, T], l = valid length, chunks of 128 ch):
  ph1   per chunk: DMA x[:, :l];  mm1 partial (PE, 512-col banks);
        DVE tensor_scalar accum -> sum x;  ACT Square accum -> sum x^2
  ph2   mu = sx/l, sd = sqrt(sx2/l - mu^2); cb = w1b@mu + w1c@sd + b1 (PE)
  ph3   h = h_ps + cb -> bf16; PE-transpose 128-blocks; bn_stats LN;
        (h-mu)*rstd per block; relu; PE-transpose back; ACT Tanh from
        PSUM -> th bf16
  ph4   per chunk: mm2 a = w2@th (PE); ACT Exp halves (accum -> den);
        DVE e*x (accum -> sex); DVE ex*x (accum -> sx2)
  ph5   mu2 = sex/den, sd2 = sqrt(sx2/den - mu2^2) -> out

Non-prefix masks fall back to a general T-wide program (the previous
kernel, kept verbatim below).
"""

import numpy as np

B, C, T = 16, 1536, 2000
ATTN = 128
NCORES = 8
BPC = B // NCORES          # samples per core
NCH = C // 128             # 12 channel chunks
MASK_NEG = -30.0

_CACHE = {}


def _ceil128(n):
    return (n + 127) // 128 * 128


def _banks(l, width=512):
    out = []
    o = 0
    while o < l:
        out.append((o, min(width, l - o)))
        o += width
    return out


def _split_waits(nc, max_waits=1):
    """walrus in this toolchain rejects >1 sync-wait per instruction; hoist
    excess waits onto injected same-engine NOPs that run just before."""
    from concourse import mybir
    ctr = 0
    for fn in nc.m.functions:
        for blk in fn.blocks:
            out = []
            changed = False
            for ins in blk.instructions:
                si = ins.sync_info
                ow = list(si.on_wait) if si and si.on_wait else []
                if len(ow) > max_waits:
                    changed = True
                    excess = ow[:-max_waits]
                    for i in range(0, len(excess), max_waits):
                        ctr += 1
                        out.append(mybir.InstNoOp(
                            name=f"wsplit_{ctr}", engine=ins.engine,
                            ins=[], outs=[],
                            sync_info=mybir.SyncInfo(
                                on_wait=excess[i:i + max_waits],
                                on_update=[])))
                    si.on_wait = ow[-max_waits:]
                    ins.sync_info = si
                out.append(ins)
            if changed:
                blk.instructions = out


def _build_nc_lens(lens, k_sq_dve=(4, 2), pool_sumx=False, pool_ph4=(False, False),
                   hT_copy_dve=True, lead_chunks=4, half_products=True,
                   pool_e2=(False, False), solo=10):
    """Length-specialized per-core program. lens = (l0, l1).

    k_sq_dve[s]: chunks per sample whose sum(x^2) runs as DVE TT+TS instead
    of ACT Square (ACT/DVE balance).  pool_sumx: run the sum(x) accums on
    the (otherwise idle) GpSimd engine.  k_ph4_pool: chunks per sample whose
    phase-4 sex accum runs on GpSimd.
    """
    import contextlib

    import concourse.bass as bass
    import concourse.tile as tile
    from concourse import mybir

    fp32 = mybir.dt.float32
    bf16 = mybir.dt.bfloat16
    AF = mybir.ActivationFunctionType
    OP = mybir.AluOpType

    lmax = max(lens)

    nc = bass.Bass("TRN2", target_bir_lowering=False)

    x_d = nc.dram_tensor("x", [BPC, C, T], bf16, kind="ExternalInput")
    w1aT_d = nc.dram_tensor("w1aT", [128, NCH, 128], bf16, kind="ExternalInput")
    w1bT_d = nc.dram_tensor("w1bT", [128, NCH, 128], bf16, kind="ExternalInput")
    w1cT_d = nc.dram_tensor("w1cT", [128, NCH, 128], bf16, kind="ExternalInput")
    w2T_d = nc.dram_tensor("w2T", [128, NCH, 128], bf16, kind="ExternalInput")
    cst_d = nc.dram_tensor("cst", [128, 16], fp32, kind="ExternalInput")
    id_d = nc.dram_tensor("ident", [128, 128], bf16, kind="ExternalInput")
    out_d = nc.dram_tensor("out", [BPC, 2 * C], fp32, kind="ExternalOutput")

    with tile.TileContext(nc) as tc:
        with contextlib.ExitStack() as ctx:
            consts = ctx.enter_context(tc.tile_pool(name="consts", bufs=1))
            xpool = ctx.enter_context(tc.tile_pool(name="xres", bufs=2))
            scr = ctx.enter_context(tc.tile_pool(name="scr", bufs=10))
            hpool = ctx.enter_context(tc.tile_pool(name="hbuf", bufs=2))
            stpool = ctx.enter_context(tc.tile_pool(name="stats", bufs=2))
            psum = ctx.enter_context(tc.tile_pool(name="ps", bufs=2, space="PSUM"))
            psa = ctx.enter_context(tc.tile_pool(name="psa", bufs=2, space="PSUM"))

            # --- constants (loaded once) ---
            w1aT = consts.tile([128, NCH, 128], bf16)
            w1bT = consts.tile([128, NCH, 128], bf16)
            w1cT = consts.tile([128, NCH, 128], bf16)
            w2T = consts.tile([128, NCH, 128], bf16)
            cst = consts.tile([128, 16], fp32)
            ident = consts.tile([128, 128], bf16)
            b1c = cst[:, 0:1]
            eps_c = cst[:, 13:14]

            LS = [int(lens[s]) for s in range(BPC)]
            LPAD = [_ceil128(l) for l in LS]
            HALVES = []
            for l in LS:
                hv = [(0, min(1024, l))]
                if l > 1024:
                    hv.append((1024, l - 1024))
                HALVES.append(hv)

            sv_t, xts, hps_t, th_t = [], [], [], []

            def sq_on_dve(s, i):
                k = k_sq_dve[s]
                if k <= 0:
                    return False
                step = NCH // k
                return i % step == 0 and i // step < k

            # ---------- phase 1: DMA emission and stats emission ----------
            for s in range(BPC):
                sv = stpool.tile([128, 360], fp32, tag="sv")
                sv_t.append(sv)
                xts.append([])

            def emit_dma(s):
                l = LS[s]
                for i in range(NCH):
                    xb = xpool.tile([128, l], bf16, tag=f"x{i}")
                    xts[s].append(xb)
                    nc.sync.dma_start(out=xb,
                                      in_=x_d[s, i * 128:(i + 1) * 128, 0:l])
                    if s == 0 and i == 2:
                        # weights after the first three x chunks: stats can
                        # start immediately; mm1 needs w1aT only once chunk
                        # 0 has landed anyway
                        for t_, d_ in ((w1aT, w1aT_d), (w1bT, w1bT_d),
                                       (w1cT, w1cT_d), (w2T, w2T_d),
                                       (cst, cst_d), (ident, id_d)):
                            nc.sync.dma_start(out=t_, in_=d_[:])

            def emit_stats(s):
                l = LS[s]
                sv = sv_t[s]
                sumx = sv[:, 144:144 + NCH]
                sumq = sv[:, 156:156 + NCH]
                for i in range(NCH):
                    xb = xts[s][i]
                    sc1 = scr.tile([128, lmax], bf16, tag="big")
                    sumx_eng = nc.gpsimd if (pool_sumx and s == 1) else nc.vector
                    sumx_eng.tensor_scalar(out=sc1[:, 0:l], in0=xb,
                                           scalar1=1.0,
                                           scalar2=0.0, op0=OP.mult,
                                           op1=OP.add,
                                           accum_out=sumx[:, i:i + 1])
                    sc2 = scr.tile([128, lmax], bf16, tag="big")
                    if sq_on_dve(s, i):
                        nc.vector.tensor_mul(out=sc2[:, 0:l], in0=xb, in1=xb)
                        nc.vector.tensor_scalar(out=sc2[:, 0:l],
                                                in0=sc2[:, 0:l],
                                                scalar1=1.0, scalar2=0.0,
                                                op0=OP.mult, op1=OP.add,
                                                accum_out=sumq[:, i:i + 1])
                    else:
                        nc.scalar.activation(out=sc2[:, 0:l], in_=xb,
                                             func=AF.Square,
                                             accum_out=sumq[:, i:i + 1])

            # ---------- per sample: mm1, ph2, LN ----------
            def emit_mid(s):
                l, lpad = LS[s], LPAD[s]
                nblk = lpad // 128
                rl = 1.0 / float(l)
                sv = sv_t[s]
                var_a = sv[:, 6 * NCH:7 * NCH]
                t0_a = sv[:, 7 * NCH:8 * NCH]
                sumx = sv[:, 144:144 + NCH]
                sumq = sv[:, 156:156 + NCH]
                cb = sv[:, 192:193]
                mv = sv[:, 208:240].rearrange("p (j two) -> p j two", two=2)
                rstd = sv[:, 240:256]
                st6 = sv[:, 256:256 + 48].rearrange("p (j k) -> p j k", k=6)
                st6b = sv[:, 144:192].rearrange("p (j k) -> p j k", k=6)

                # mm1 in 1024-col halves so each PSUM accumulator is 2 banks
                h_ps = []
                hps_t.append(h_ps)
                for (ho, hn) in HALVES[s]:
                    hp = psum.tile([128, 1024], fp32, tag="mm")
                    h_ps.append((ho, hn, hp))
                    for i in range(NCH):
                        for (o, n) in _banks(hn):
                            nc.tensor.matmul(
                                hp[:, o:o + n],
                                lhsT=w1aT[:, i, :],
                                rhs=xts[s][i][:, ho + o:ho + o + n],
                                start=(i == 0), stop=(i == NCH - 1))

                # ---------- phase 2: mu/sd (bf16, short chain), cb ----------
                mu_b = stpool.tile([128, 2 * NCH], bf16, tag="mub")
                nc.vector.tensor_scalar(out=mu_b[:, 0:NCH], in0=sumx,
                                        scalar1=rl, scalar2=None, op0=OP.mult)
                nc.vector.tensor_mul(out=t0_a, in0=mu_b[:, 0:NCH],
                                     in1=mu_b[:, 0:NCH])
                nc.vector.scalar_tensor_tensor(out=var_a, in0=sumq, scalar=rl,
                                               in1=t0_a, op0=OP.mult,
                                               op1=OP.subtract)
                nc.vector.tensor_scalar(out=var_a, in0=var_a, scalar1=1e-9,
                                        scalar2=None, op0=OP.max)
                nc.scalar.activation(out=mu_b[:, NCH:2 * NCH], in_=var_a,
                                     func=AF.Sqrt)
                cb_ps = psa.tile([128, 2048], bf16, tag="a")
                cb_ps32 = cb_ps.bitcast(fp32)
                for i in range(NCH):
                    nc.tensor.matmul(cb_ps32[:, 0:1], lhsT=w1bT[:, i, :],
                                     rhs=mu_b[:, i:i + 1],
                                     start=(i == 0), stop=False)
                for i in range(NCH):
                    nc.tensor.matmul(cb_ps32[:, 0:1], lhsT=w1cT[:, i, :],
                                     rhs=mu_b[:, NCH + i:NCH + i + 1],
                                     start=False, stop=(i == NCH - 1))
                nc.vector.tensor_tensor(out=cb, in0=cb_ps32[:, 0:1],
                                         in1=b1c, op=OP.add)

                # ---------- phase 3: h -> LN -> relu -> (T) tanh -> th ------
                h_sb = hpool.tile([128, lpad], bf16, tag="hsb")
                for (ho, hn, hp) in h_ps:
                    nc.scalar.activation(out=h_sb[:, ho:ho + hn],
                                         in_=hp[:, 0:hn],
                                         func=AF.Identity, bias=cb)
                if lpad > l:
                    nc.vector.memset(h_sb[:, l:lpad], 0.0)

                tp_ps = psum.tile([128, 2048], bf16, tag="mm")
                for j in range(nblk):
                    nc.tensor.transpose(
                        tp_ps[:, j * 128:(j + 1) * 128],
                        in_=h_sb[:, j * 128:(j + 1) * 128],
                        identity=ident)

                for j in range(nblk):
                    stt = st6 if j < 8 else st6b
                    nc.vector.bn_stats(out=stt[:, j % 8, :],
                                       in_=tp_ps[:, j * 128:(j + 1) * 128])
                    nc.vector.bn_aggr(out=mv[:, j, :], in_=stt[:, j % 8, :])
                for (g0, g1) in ((0, min(8, nblk)), (8, nblk)):
                    if g0 >= g1:
                        continue
                    nc.scalar.activation(out=rstd[:, g0:g1],
                                         in_=mv[:, g0:g1, 1],
                                         func=AF.Sqrt, bias=eps_c)
                    nc.vector.reciprocal(out=rstd[:, g0:g1],
                                         in_=rstd[:, g0:g1])

                thT = hpool.tile([128, lpad], bf16, tag="thT")
                for j in range(nblk):
                    blk = slice(j * 128, (j + 1) * 128)
                    nc.vector.tensor_scalar(
                        out=thT[:, blk], in0=tp_ps[:, blk],
                        scalar1=mv[:, j, 0:1], scalar2=rstd[:, j:j + 1],
                        op0=OP.subtract, op1=OP.mult)
                nc.vector.tensor_scalar(out=thT, in0=thT, scalar1=0.0,
                                        scalar2=None, op0=OP.max)

                th = hpool.tile([128, lpad], bf16, tag="th")
                th_t.append(th)
                tb_ps = psum.tile([128, 2048], bf16, tag="mm")
                for j in range(nblk):
                    nc.tensor.transpose(
                        tb_ps[:, j * 128:(j + 1) * 128],
                        in_=thT[:, j * 128:(j + 1) * 128],
                        identity=ident)
                for (co, cn) in _banks(lpad, 512):
                    nc.scalar.activation(out=th[:, co:co + cn],
                                         in_=tb_ps[:, co:co + cn],
                                         func=AF.Tanh)

            def emit_ph4_chunk(s, i, use_mm=False):
                l = LS[s]
                sv = sv_t[s]
                denh = sv[:, 168:168 + 2 * NCH]
                sexh = sv[:, 304:304 + 2 * NCH]
                sx2h = sv[:, 328:328 + 2 * NCH]
                if s == 0:
                    acc1 = nc.gpsimd if (pool_ph4[0] and i % 2 == 0) else nc.vector
                    acc2 = nc.gpsimd if pool_ph4[0] else nc.vector
                else:
                    acc1 = nc.gpsimd if (pool_ph4[1] and i >= 6) else nc.vector
                    acc2 = nc.vector
                e_i = scr.tile([128, lmax], bf16, tag="big")
                ex_i = scr.tile([128, lmax], bf16, tag="big")
                e2_i = scr.tile([128, lmax], bf16, tag="big")
                for hf, (ho, hn) in enumerate(HALVES[s]):
                    if use_mm and hf % 2 == 1:
                        a_ps = psum.tile([128, 2048], bf16, tag="mm")
                    else:
                        a_ps = psa.tile([128, 2048], bf16, tag="a")
                    a_ps32 = a_ps.bitcast(fp32)
                    for (o, n) in _banks(hn):
                        nc.tensor.matmul(
                            a_ps32[:, o:o + n],
                            lhsT=w2T[:, i, :],
                            rhs=th_t[s][:, ho + o:ho + o + n],
                            start=True, stop=True)
                    nc.scalar.activation(
                        out=e_i[:, ho:ho + hn], in_=a_ps32[:, 0:hn],
                        func=AF.Exp,
                        accum_out=denh[:, hf * NCH + i:hf * NCH + i + 1])
                    if not half_products:
                        continue
                    hsl = slice(ho, ho + hn)
                    nc.vector.tensor_mul(out=ex_i[:, hsl], in0=e_i[:, hsl],
                                         in1=xts[s][i][:, hsl])
                    mul2 = nc.gpsimd if pool_e2[s] else nc.vector
                    mul2.tensor_mul(out=e2_i[:, hsl], in0=ex_i[:, hsl],
                                    in1=xts[s][i][:, hsl])
                    # e_i half is dead after the first product; reuse it as
                    # the accum-pass output so nothing serializes on ex_i
                    acc1.tensor_scalar(out=ex_i[:, hsl], in0=ex_i[:, hsl],
                                       scalar1=1.0, scalar2=0.0,
                                       op0=OP.mult, op1=OP.add,
                                       accum_out=sexh[:, hf * NCH + i:
                                                      hf * NCH + i + 1])
                    acc2.tensor_scalar(out=e2_i[:, hsl], in0=e2_i[:, hsl],
                                       scalar1=1.0, scalar2=0.0,
                                       op0=OP.mult, op1=OP.add,
                                       accum_out=sx2h[:, hf * NCH + i:
                                                      hf * NCH + i + 1])
                if not half_products:
                    sl = slice(0, l)
                    nc.vector.tensor_mul(out=ex_i[:, sl], in0=e_i[:, sl],
                                         in1=xts[s][i])
                    mul2 = nc.gpsimd if pool_e2[s] else nc.vector
                    mul2.tensor_mul(out=e2_i[:, sl], in0=ex_i[:, sl],
                                    in1=xts[s][i])
                    acc1.tensor_scalar(out=ex_i[:, sl], in0=ex_i[:, sl],
                                       scalar1=1.0, scalar2=0.0,
                                       op0=OP.mult, op1=OP.add,
                                       accum_out=sexh[:, i:i + 1])
                    acc2.tensor_scalar(out=e2_i[:, sl], in0=e2_i[:, sl],
                                       scalar1=1.0, scalar2=0.0,
                                       op0=OP.mult, op1=OP.add,
                                       accum_out=sx2h[:, i:i + 1])

            # emission order: s0 dma+stats, s1 dma, s0 mid-pipe, s1 stats,
            # lead chunks of ph4(s0), s1 mid-pipe, interleaved remainder
            emit_dma(0)
            emit_stats(0)
            emit_dma(1)
            with tc.high_priority(offset=100000):
                emit_mid(0)
            emit_stats(1)
            with tc.high_priority(offset=70000):
                for i in range(min(lead_chunks, NCH)):
                    emit_ph4_chunk(0, i)
            emit_mid(1)
            # s0-solo prefix, then 1:1 alternate, then s1-only tail: s1 work
            # enters the in-order engine streams only once LN(s1) output
    # exists, avoiding head-of-line stalls
            seq = [(0, i) for i in range(lead_chunks, solo)]
            j1 = 0
            for i in range(solo, NCH):
                seq.append((0, i))
                seq.append((1, j1))
                j1 += 1
            while j1 < NCH:
                seq.append((1, j1))
                j1 += 1
            for k, (s, i) in enumerate(seq):
                emit_ph4_chunk(s, i, use_mm=k % 2 == 1)

            # ---------- phase 5: outputs ----------
            for s in range(BPC):
                sv = sv_t[s]
                den_a = sv[:, 3 * NCH:4 * NCH]
                sex_a = sv[:, 4 * NCH:5 * NCH]
                sx2_a = sv[:, 5 * NCH:6 * NCH]
                t0_a = sv[:, 7 * NCH:8 * NCH]
                mu2_a = sv[:, 8 * NCH:9 * NCH]
                ms2_a = sv[:, 9 * NCH:10 * NCH]
                sd2_a = sv[:, 10 * NCH:11 * NCH]
                rden_a = sv[:, 11 * NCH:12 * NCH]
                denh = sv[:, 168:168 + 2 * NCH]
                sexh = sv[:, 304:304 + 2 * NCH]
                sx2h = sv[:, 328:328 + 2 * NCH]
                if len(HALVES[s]) > 1:
                    nc.vector.tensor_add(out=den_a, in0=denh[:, 0:NCH],
                                         in1=denh[:, NCH:2 * NCH])
                else:
                    nc.vector.tensor_scalar(out=den_a, in0=denh[:, 0:NCH],
                                            scalar1=1.0, scalar2=None,
                                            op0=OP.mult)
                if len(HALVES[s]) > 1 and half_products:
                    nc.vector.tensor_add(out=sex_a, in0=sexh[:, 0:NCH],
                                         in1=sexh[:, NCH:2 * NCH])
                    nc.vector.tensor_add(out=sx2_a, in0=sx2h[:, 0:NCH],
                                         in1=sx2h[:, NCH:2 * NCH])
                else:
                    nc.vector.tensor_scalar(out=sex_a, in0=sexh[:, 0:NCH],
                                            scalar1=1.0, scalar2=None,
                                            op0=OP.mult)
                    nc.vector.tensor_scalar(out=sx2_a, in0=sx2h[:, 0:NCH],
                                            scalar1=1.0, scalar2=None,
                                            op0=OP.mult)
                nc.vector.reciprocal(out=rden_a, in_=den_a)
                nc.vector.tensor_mul(out=mu2_a, in0=sex_a, in1=rden_a)
                nc.vector.tensor_mul(out=ms2_a, in0=sx2_a, in1=rden_a)
                nc.vector.tensor_mul(out=t0_a, in0=mu2_a, in1=mu2_a)
                nc.vector.tensor_tensor(out=ms2_a, in0=ms2_a, in1=t0_a,
                                        op=OP.subtract)
                nc.vector.tensor_scalar(out=ms2_a, in0=ms2_a, scalar1=1e-9,
                                        scalar2=None, op0=OP.max)
                nc.scalar.activation(out=sd2_a, in_=ms2_a, func=AF.Sqrt)

                nc.sync.dma_start(
                    out=out_d[s, 0:C].rearrange("(i p) -> p i", p=128),
                    in_=mu2_a)
                nc.sync.dma_start(
                    out=out_d[s, C:2 * C].rearrange("(i p) -> p i", p=128),
                    in_=sd2_a)

    _split_waits(nc)
    return nc


def _prep_weights_v3(w1, b1, w2):
    import ml_dtypes
    f = np.float32
    bf = ml_dtypes.bfloat16
    w1T = np.ascontiguousarray(np.asarray(w1, f).T)      # [3C, 128]
    w1aT = np.ascontiguousarray(
        w1T[0:C].reshape(NCH, 128, 128).transpose(1, 0, 2)).astype(bf)
    w1bT = np.ascontiguousarray(
        w1T[C:2 * C].reshape(NCH, 128, 128).transpose(1, 0, 2)).astype(bf)
    w1cT = np.ascontiguousarray(
        w1T[2 * C:3 * C].reshape(NCH, 128, 128).transpose(1, 0, 2)).astype(bf)
    w2T = np.ascontiguousarray(
        np.asarray(w2, f).reshape(NCH, 128, 128).transpose(2, 0, 1)).astype(bf)
    cst = np.zeros((128, 16), f)
    cst[:, 0] = np.asarray(b1, f)
    cst[:, 13] = 1e-5
    ident = np.eye(128, dtype=bf)
    return dict(w1aT=w1aT, w1bT=w1bT, w1cT=w1cT, w2T=w2T, cst=cst,
                ident=ident)


def _run_programs_on_devices(progs, in_maps):
    """Run per-core Bass programs, program i pinned to jax device i.

    Mirrors bass2jax.run_bass_via_pjrt's single-core path, with explicit
    device placement so the 8 distinct programs land on 8 distinct cores.
    """
    import jax
    from concourse import bass2jax, mybir

    bass2jax.install_neuronx_cc_hook()
    devices = jax.devices()
    futs = []
    metas = []
    for core, (nc, in_map) in enumerate(zip(progs, in_maps)):
        part_name = (nc.partition_id_tensor.name
                     if nc.partition_id_tensor else None)
        in_names, out_names, out_avals, zero_outs = [], [], [], []
        for alloc in nc.m.functions[0].allocations:
            if not isinstance(alloc, mybir.MemoryLocationSet):
                continue
            name = alloc.memorylocations[0].name
            if alloc.kind == "ExternalInput":
                if name != part_name:
                    in_names.append(name)
            elif alloc.kind == "ExternalOutput":
                out_names.append(name)
                shape = tuple(alloc.tensor_shape)
                dtype = mybir.dt.np(alloc.dtype)
                out_avals.append(jax.core.ShapedArray(shape, dtype))
                zero_outs.append(np.zeros(shape, dtype))
        n_params = len(in_names)
        all_names = list(in_names) + list(out_names)
        if part_name is not None:
            all_names.append(part_name)
        all_names = tuple(all_names)
        donate = tuple(range(n_params, n_params + len(out_names)))

        def _body(*args, _nc=nc, _avals=tuple(out_avals), _names=all_names,
                  _onames=tuple(out_names), _part=part_name):
            operands = list(args)
            if _part is not None:
                operands.append(bass2jax.partition_id_tensor())
            outs = bass2jax._bass_exec_p.bind(
                *operands,
                out_avals=_avals,
                in_names=_names,
                out_names=_onames,
                lowering_input_output_aliases=(),
                sim_require_finite=True,
                sim_require_nnan=True,
                nc=_nc,
            )
            return tuple(outs)

        args = [np.asarray(in_map[nm]) for nm in in_names] + zero_outs
        dev = devices[core % len(devices)]
        with jax.default_device(dev):
            out_arrs = jax.jit(_body, donate_argnums=donate,
                               keep_unused=True)(*args)
        futs.append(out_arrs)
        metas.append(out_names)
    results = []
    for out_arrs, out_names in zip(futs, metas):
        results.append({nm: np.asarray(a) for nm, a in zip(out_names, out_arrs)})
    return results


def _kernel_prefix(x, mask, lens, w1, b1, w2, b2):
    import ml_dtypes

    order = np.argsort(-lens, kind="stable")          # longest first
    pairs = [(int(order[i]), int(order[B - 1 - i])) for i in range(NCORES)]

    wts = _prep_weights_v3(w1, b1, w2)
    xf = np.asarray(x, np.float32).astype(ml_dtypes.bfloat16)

    progs, in_maps = [], []
    for (sa, sb) in pairs:
        key = ("lens", int(lens[sa]), int(lens[sb]))
        if key not in _CACHE:
            _CACHE[key] = _build_nc_lens((int(lens[sa]), int(lens[sb])))
        progs.append(_CACHE[key])
        m = {"x": np.ascontiguousarray(xf[[sa, sb]])}
        m.update(wts)
        in_maps.append(m)

    res = _run_programs_on_devices(progs, in_maps)
    out = np.zeros((B, 2 * C), np.float32)
    for ci, (sa, sb) in enumerate(pairs):
        o = res[ci]["out"].reshape(BPC, 2 * C)
        out[sa] = o[0]
        out[sb] = o[1]
    _CACHE["last_progs"] = progs
    return out


# ---------------------------------------------------------------------------
# General fallback (arbitrary 0/1 masks): previous full-T kernel, verbatim.
# ---------------------------------------------------------------------------

TPAD = 2048
NTT = TPAD // 128
BANKS = [(0, 512), (512, 512), (1024, 512), (1536, 464)]


def _build_nc_general(trivial_ln=True, trivial_b2=True):
    import concourse.bass as bass
    import concourse.tile as tile
    from concourse import mybir

    fp32 = mybir.dt.float32
    AF = mybir.ActivationFunctionType
    OP = mybir.AluOpType

    nc = bass.Bass("TRN2", target_bir_lowering=False)

    bf16 = mybir.dt.bfloat16
    x_d = nc.dram_tensor("x", [BPC, C, T], bf16, kind="ExternalInput")
    mk_d = nc.dram_tensor("maskb", [BPC, T], bf16, kind="ExternalInput")
    mbias_d = nc.dram_tensor("mbias", [BPC, T], bf16, kind="ExternalInput")
    w1aT_d = nc.dram_tensor("w1aT", [128, NCH, 128], bf16, kind="ExternalInput")
    w1bT_d = nc.dram_tensor("w1bT", [128, NCH, 128], fp32, kind="ExternalInput")
    w1cT_d = nc.dram_tensor("w1cT", [128, NCH, 128], fp32, kind="ExternalInput")
    w2T_d = nc.dram_tensor("w2T", [128, NCH, 128], bf16, kind="ExternalInput")
    gb_d = nc.dram_tensor("gb", [128, 128], fp32, kind="ExternalInput")
    bb_d = nc.dram_tensor("bb", [128, 128], fp32, kind="ExternalInput")
    cst_d = nc.dram_tensor("cst", [128, 16], fp32, kind="ExternalInput")
    ones_d = nc.dram_tensor("ones_row", [1, 128], bf16, kind="ExternalInput")
    id_d = nc.dram_tensor("ident", [128, 128], fp32, kind="ExternalInput")
    out_d = nc.dram_tensor("out", [BPC, 2 * C], fp32, kind="ExternalOutput")

    with tile.TileContext(nc) as tc:
        import contextlib
        with contextlib.ExitStack() as ctx:
            consts = ctx.enter_context(tc.tile_pool(name="consts", bufs=1))
            xpool = ctx.enter_context(tc.tile_pool(name="xres", bufs=1))
            nmpool = ctx.enter_context(tc.tile_pool(name="nm", bufs=2))
            scr = ctx.enter_context(tc.tile_pool(name="scr", bufs=4))
            hpool = ctx.enter_context(tc.tile_pool(name="hbuf", bufs=2))
            stpool = ctx.enter_context(tc.tile_pool(name="stats", bufs=2))
            psum = ctx.enter_context(tc.tile_pool(name="ps", bufs=1, space="PSUM"))
            psa = ctx.enter_context(tc.tile_pool(name="psa", bufs=2, space="PSUM"))

            w1aT = consts.tile([128, NCH, 128], bf16)
            w1bT = consts.tile([128, NCH, 128], fp32)
            w1cT = consts.tile([128, NCH, 128], fp32)
            w2T = consts.tile([128, NCH, 128], bf16)
            gb = consts.tile([128, 128], fp32)
            bb = consts.tile([128, 128], fp32)
            cst = consts.tile([128, 16], fp32)
            ones_row = consts.tile([1, 128], bf16)
            ident = consts.tile([128, 128], fp32)
            for t_, d_ in ((w1aT, w1aT_d), (w1bT, w1bT_d), (w1cT, w1cT_d),
                           (w2T, w2T_d), (gb, gb_d), (bb, bb_d), (cst, cst_d),
                           (ones_row, ones_d), (ident, id_d)):
                nc.sync.dma_start(out=t_, in_=d_[:])
            b1c = cst[:, 0:1]
            b2c = cst[:, 1:1 + NCH]
            eps_c = cst[:, 13:14]

            for s in range(BPC):
                mb_row = stpool.tile([1, T], bf16, tag="mbrow")
                nc.sync.dma_start(out=mb_row, in_=mbias_d[s, :][None, :])

                sv = stpool.tile([128, 360], fp32, tag="sv")
                mu_a = sv[:, 0 * NCH:1 * NCH]
                q_a = sv[:, 1 * NCH:2 * NCH]
                sd_a = sv[:, 2 * NCH:3 * NCH]
                den_a = sv[:, 3 * NCH:4 * NCH]
                sex_a = sv[:, 4 * NCH:5 * NCH]
                sx2_a = sv[:, 5 * NCH:6 * NCH]
                var_a = sv[:, 6 * NCH:7 * NCH]
                t0_a = sv[:, 7 * NCH:8 * NCH]
                mu2_a = sv[:, 8 * NCH:9 * NCH]
                ms2_a = sv[:, 9 * NCH:10 * NCH]
                sd2_a = sv[:, 10 * NCH:11 * NCH]
                rden_a = sv[:, 11 * NCH:12 * NCH]
                msum = sv[0:1, 144:145]
                L_b = sv[:, 145:146]
                rcpL_b = sv[:, 146:147]
                cb = sv[:, 147:148]
                mv = sv[:, 160:192].rearrange("p (j two) -> p j two", two=2)
                rstd = sv[:, 192:208]
                st6 = sv[:, 208:304].rearrange("p (j k) -> p j k", k=6)

                nm_b = nmpool.tile([128, T], bf16, tag="nmb")
                nc.sync.dma_start(
                    out=nm_b, in_=mk_d[s, :][None, :].to_broadcast((128, T)))
                nc.scalar.activation(out=nm_b, in_=nm_b, func=AF.Copy,
                                     accum_out=L_b)
                nc.vector.reciprocal(out=rcpL_b, in_=L_b)
                nc.vector.tensor_scalar(out=nm_b, in0=nm_b, scalar1=rcpL_b,
                                        scalar2=None, op0=OP.mult)

                h_ps = psum.tile([128, TPAD], fp32, tag="mm")
                xt = []
                for i in range(NCH):
                    xb = xpool.tile([128, T], bf16, tag=f"x{i}")
                    xt.append(xb)
                    nc.sync.dma_start(out=xb, in_=x_d[s, i * 128:(i + 1) * 128, :])
                    for (o, n) in BANKS:
                        nc.tensor.matmul(
                            h_ps[:, o:o + n],
                            lhsT=w1aT[:, i, :],
                            rhs=xb[:, o:o + n],
                            start=(i == 0), stop=(i == NCH - 1))
                    xnm = scr.tile([128, T], bf16, tag="big")
                    nc.vector.tensor_mul(out=xnm, in0=xb, in1=nm_b)
                    nc.vector.tensor_scalar(out=xnm, in0=xnm, scalar1=1.0,
                                            scalar2=0.0, op0=OP.mult,
                                            op1=OP.add,
                                            accum_out=mu_a[:, i:i + 1])
                    nc.scalar.activation(out=xnm, in_=xnm, func=AF.Square,
                                         accum_out=q_a[:, i:i + 1])

                nc.vector.tensor_scalar(out=q_a, in0=q_a, scalar1=L_b,
                                        scalar2=None, op0=OP.mult)
                nc.vector.tensor_mul(out=t0_a, in0=mu_a, in1=mu_a)
                nc.vector.tensor_tensor(out=var_a, in0=q_a, in1=t0_a,
                                        op=OP.subtract)
                nc.vector.tensor_scalar(out=var_a, in0=var_a, scalar1=1e-9,
                                        scalar2=None, op0=OP.max)
                nc.scalar.activation(out=sd_a, in_=var_a, func=AF.Sqrt)

                cb_ps = psa.tile([128, 1], fp32, tag="a")
                for i in range(NCH):
                    nc.tensor.matmul(cb_ps, lhsT=w1bT[:, i, :],
                                     rhs=mu_a[:, i:i + 1],
                                     start=(i == 0), stop=False)
                for i in range(NCH):
                    nc.tensor.matmul(cb_ps, lhsT=w1cT[:, i, :],
                                     rhs=sd_a[:, i:i + 1],
                                     start=False, stop=(i == NCH - 1))
                nc.scalar.activation(out=cb, in_=cb_ps, func=AF.Identity,
                                     bias=b1c)

                h_sb = hpool.tile([128, TPAD], fp32, tag="hbuf")
                nc.scalar.activation(out=h_sb[:, 0:T], in_=h_ps[:, 0:T],
                                     func=AF.Identity, bias=cb)
                nc.vector.memset(h_sb[:, T:TPAD], 0.0)

                hT = hpool.tile([128, TPAD], fp32, tag="hbuf")
                for g in range(2):
                    tp_ps = psa.tile([128, 1024], fp32, tag="a")
                    for j in range(8):
                        jj = g * 8 + j
                        nc.tensor.transpose(tp_ps[:, j * 128:(j + 1) * 128],
                                            in_=h_sb[:, jj * 128:(jj + 1) * 128],
                                            identity=ident)
                    nc.scalar.activation(out=hT[:, g * 1024:(g + 1) * 1024],
                                         in_=tp_ps, func=AF.Copy)

                for j in range(NTT):
                    nc.vector.bn_stats(out=st6[:, j, :],
                                       in_=hT[:, j * 128:(j + 1) * 128])
                    nc.vector.bn_aggr(out=mv[:, j, :], in_=st6[:, j, :])
                nc.scalar.activation(out=rstd, in_=mv[:, :, 1], func=AF.Sqrt,
                                     bias=eps_c)
                nc.vector.reciprocal(out=rstd, in_=rstd)

                thT = hpool.tile([128, TPAD], fp32, tag="hbuf")
                for j in range(NTT):
                    blk = slice(j * 128, (j + 1) * 128)
                    nc.vector.tensor_scalar(
                        out=thT[:, blk], in0=hT[:, blk],
                        scalar1=mv[:, j, 0:1], scalar2=rstd[:, j:j + 1],
                        op0=OP.subtract, op1=OP.mult)
                    if not trivial_ln:
                        nc.vector.tensor_mul(out=thT[:, blk],
                                             in0=thT[:, blk], in1=gb)
                        nc.vector.tensor_add(out=thT[:, blk],
                                             in0=thT[:, blk], in1=bb)
                nc.vector.tensor_scalar(out=thT, in0=thT, scalar1=0.0,
                                        scalar2=None, op0=OP.max)
                nc.scalar.activation(out=thT, in_=thT, func=AF.Tanh)

                th = hpool.tile([128, TPAD], bf16, tag="thbuf")
                for g in range(2):
                    tb_ps = psa.tile([128, 1024], fp32, tag="a")
                    for j in range(8):
                        jj = g * 8 + j
                        nc.tensor.transpose(tb_ps[:, j * 128:(j + 1) * 128],
                                            in_=thT[:, jj * 128:(jj + 1) * 128],
                                            identity=ident)
                    nc.scalar.activation(out=th[:, g * 1024:(g + 1) * 1024],
                                         in_=tb_ps, func=AF.Copy)

                denh = stpool.tile([128, 2 * NCH], fp32, tag="denh")
                for i in range(NCH):
                    e_i = scr.tile([128, T], bf16, tag="big")
                    for hf, (ho, hws) in enumerate(
                            ((0, ((0, 512), (512, 512))),
                             (1024, ((0, 512), (512, 464))))):
                        a_ps = psa.tile([128, 1024], fp32, tag="a")
                        for (o, n) in hws:
                            nc.tensor.matmul(
                                a_ps[:, o:o + n],
                                lhsT=ones_row,
                                rhs=mb_row[:, ho + o:ho + o + n],
                                start=True, stop=False)
                            nc.tensor.matmul(
                                a_ps[:, o:o + n],
                                lhsT=w2T[:, i, :],
                                rhs=th[:, ho + o:ho + o + n],
                                start=False, stop=True)
                        hn = 1024 if hf == 0 else T - 1024
                        kw = {} if trivial_b2 else {"bias": b2c[:, i:i + 1]}
                        nc.scalar.activation(
                            out=e_i[:, ho:ho + hn], in_=a_ps[:, 0:hn],
                            func=AF.Exp,
                            accum_out=denh[:, hf * NCH + i:hf * NCH + i + 1],
                            **kw)
                    ex_i = scr.tile([128, T], bf16, tag="big")
                    nc.vector.tensor_mul(out=ex_i, in0=e_i, in1=xt[i])
                    nc.vector.tensor_scalar(out=ex_i, in0=ex_i, scalar1=1.0,
                                            scalar2=0.0, op0=OP.mult,
                                            op1=OP.add,
                                            accum_out=sex_a[:, i:i + 1])
                    nc.vector.tensor_mul(out=e_i, in0=ex_i, in1=xt[i])
                    nc.vector.tensor_scalar(out=e_i, in0=e_i, scalar1=1.0,
                                            scalar2=0.0, op0=OP.mult,
                                            op1=OP.add,
                                            accum_out=sx2_a[:, i:i + 1])

                nc.vector.tensor_add(out=den_a, in0=denh[:, 0:NCH],
                                     in1=denh[:, NCH:2 * NCH])
                nc.vector.reciprocal(out=rden_a, in_=den_a)
                nc.vector.tensor_mul(out=mu2_a, in0=sex_a, in1=rden_a)
                nc.vector.tensor_mul(out=ms2_a, in0=sx2_a, in1=rden_a)
                nc.vector.tensor_mul(out=t0_a, in0=mu2_a, in1=mu2_a)
                nc.vector.tensor_tensor(out=ms2_a, in0=ms2_a, in1=t0_a,
                                        op=OP.subtract)
                nc.vector.tensor_scalar(out=ms2_a, in0=ms2_a, scalar1=1e-9,
                                        scalar2=None, op0=OP.max)
                nc.scalar.activation(out=sd2_a, in_=ms2_a, func=AF.Sqrt)

                nc.sync.dma_start(
                    out=out_d[s, 0:C].rearrange("(i p) -> p i", p=128),
                    in_=mu2_a)
                nc.sync.dma_start(
                    out=out_d[s, C:2 * C].rearrange("(i p) -> p i", p=128),
                    in_=sd2_a)

    _split_waits(nc)
    return nc


def _prep_weights_general(w1, b1, ln_g, ln_b, w2, b2):
    f = np.float32
    import ml_dtypes
    bf = ml_dtypes.bfloat16
    w1T = np.ascontiguousarray(w1.T, dtype=f)
    w1aT = np.ascontiguousarray(
        w1T[0:C].reshape(NCH, 128, 128).transpose(1, 0, 2)).astype(bf)
    w1bT = np.ascontiguousarray(
        w1T[C:2 * C].reshape(NCH, 128, 128).transpose(1, 0, 2))
    w1cT = np.ascontiguousarray(
        w1T[2 * C:3 * C].reshape(NCH, 128, 128).transpose(1, 0, 2))
    w2T = np.ascontiguousarray(
        np.asarray(w2, f).reshape(NCH, 128, 128).transpose(2, 0, 1)).astype(bf)
    gb = np.ascontiguousarray(np.tile(np.asarray(ln_g, f)[None, :], (128, 1)))
    bb = np.ascontiguousarray(np.tile(np.asarray(ln_b, f)[None, :], (128, 1)))
    cst = np.zeros((128, 16), f)
    cst[:, 0] = np.asarray(b1, f)
    cst[:, 1:1 + NCH] = np.asarray(b2, f).reshape(NCH, 128).T
    cst[:, 13] = 1e-5
    ones_row = np.ones((1, 128), bf)
    ident = np.eye(128, dtype=f)
    return dict(w1aT=w1aT, w1bT=w1bT, w1cT=w1cT, w2T=w2T, gb=gb, bb=bb,
                cst=cst, ones_row=ones_row, ident=ident)


def _kernel_general(x, mask, w1, b1, ln_g, ln_b, w2, b2):
    from concourse.bass_utils import run_bass_kernel_spmd

    trivial_ln = bool(np.all(np.asarray(ln_g) == 1.0)
                      and np.all(np.asarray(ln_b) == 0.0))
    trivial_b2 = bool(np.all(np.asarray(b2) == 0.0))
    key = ("gen", trivial_ln, trivial_b2)
    if key not in _CACHE:
        _CACHE[key] = _build_nc_general(trivial_ln, trivial_b2)
    nc = _CACHE[key]

    wts = _prep_weights_general(w1, b1, ln_g, ln_b, w2, b2)
    import ml_dtypes
    xf = np.ascontiguousarray(
        np.asarray(x, np.float32).astype(ml_dtypes.bfloat16))
    mf = np.ascontiguousarray(np.asarray(mask, np.float32).reshape(B, T))
    maskb = np.ascontiguousarray(mf.astype(ml_dtypes.bfloat16))
    mbias = np.ascontiguousarray(
        ((mf - 1.0) * -MASK_NEG).astype(ml_dtypes.bfloat16))

    in_maps = []
    for c in range(NCORES):
        m = {"x": xf[c * BPC:(c + 1) * BPC],
             "maskb": maskb[c * BPC:(c + 1) * BPC],
             "mbias": mbias[c * BPC:(c + 1) * BPC]}
        m.update(wts)
        in_maps.append(m)

    res = run_bass_kernel_spmd(nc, in_maps, list(range(NCORES)))
    out = np.concatenate([res.results[c]["out"] for c in range(NCORES)], axis=0)
    _CACHE["last_progs"] = [nc]
    return out.reshape(B, 2 * C)


def kernel(x, mask, w1, b1, ln_g, ln_b, w2, b2):
    mask2 = np.asarray(mask).reshape(B, T)
    lens = mask2.sum(axis=-1).astype(np.int64)
    is_prefix = bool(
        np.all((np.arange(T)[None, :] < lens[:, None]) == (mask2 != 0)))
    trivial_ln = bool(np.all(np.asarray(ln_g) == 1.0)
                      and np.all(np.asarray(ln_b) == 0.0))
    trivial_b2 = bool(np.all(np.asarray(b2) == 0.0))
    if is_prefix and trivial_ln and trivial_b2 and np.all(lens >= 1):
        return _kernel_prefix(x, mask2, lens, w1, b1, w2, b2)
    return _kernel_general(x, mask, w1, b1, ln_g, ln_b, w2, b2)
